# revision 46
# baseline (speedup 1.0000x reference)
"""Trainium2 Bass kernel for nn_AFRM_1245540516473.

Bidirectional 4-step LSTM (hidden 1024, with the module's state-swap bug)
over (B=4096, 4, 4, 256) NHWC images, followed by a 1x1 conv (512->256),
LeakyReLU(0.3), and residual:  out = gamma * lrelu(conv(cat(hs_f, hs_r))) + x.

Strategy: pure data parallelism over 8 NeuronCores (512 batch rows per
core), weights replicated.  Host pre-packing: [W;U] stacks quantized to
fp8 e4m3 in DoubleRow pair layout with gate columns regrouped, the reverse
direction's channel flip absorbed into Wr's row order, and the hidden
dimension permuted (p' = hh*256+cc) so the 1x1 conv contracts contiguous
partition chunks.  On device, z is computed transposed (gate columns on
partitions, batch rows free) via fp8 DoubleRow matmuls (K=256/matmul)
streaming weights once per step, so the recurrent state leaves the cell in
exactly the layout the next step's matmuls need; the two directions
interleave at chunk granularity, the per-step conv + fused epilogue
(gamma*lrelu(y)+x as (0.3g)y + (0.7g)relu(y) + x — relu shares the ACT LUT
set with sigmoid/tanh, so no table reloads) slot in as soon as their hidden
chunks are ready, and gate/conv PSUM tiles share one 8-bank pool.
"""

import os
import sys

for _p in ("/opt/trn_rl_repo",):
    if _p not in sys.path and os.path.isdir(_p):
        sys.path.insert(0, _p)

import numpy as np
import ml_dtypes


def _install_ntff_hook():
    """Provide antenv.axon_hooks (absent in this image) so
    run_bass_kernel_spmd(trace=True) can capture NTFF profiles through
    libaxon_pjrt.so. No-op when profiling isn't requested or available."""
    import types
    import ctypes
    import contextlib
    try:
        import antenv
    except ImportError:
        return
    if "antenv.axon_hooks" in sys.modules:
        return
    state = {}
    mod = types.ModuleType("antenv.axon_hooks")
    mod.set_axon_ntff_profile_hook = lambda h: state.__setitem__("h", h)
    mod.get_axon_ntff_profile_hook = lambda: state.get("h")
    sys.modules["antenv.axon_hooks"] = mod
    antenv.axon_hooks = mod

    so_path = "/opt/axon/libaxon_pjrt.so"
    if not os.path.exists(so_path):
        return
    lib = ctypes.CDLL(so_path)
    if not hasattr(lib, "axon_start_nrt_profile"):
        return
    lib.axon_start_nrt_profile.argtypes = [ctypes.POINTER(ctypes.c_int64), ctypes.c_size_t]
    lib.axon_start_nrt_profile.restype = ctypes.c_int64
    lib.axon_stop_nrt_profile.argtypes = [ctypes.c_char_p]
    lib.axon_stop_nrt_profile.restype = ctypes.c_int64

    @contextlib.contextmanager
    def _hook(output_dir, device_ids):
        import jax
        jax.devices()
        if device_ids:
            ids = (ctypes.c_int64 * len(device_ids))(*device_ids)
            rc = lib.axon_start_nrt_profile(ids, len(device_ids))
        else:
            rc = lib.axon_start_nrt_profile(None, 0)
        if rc != 0:
            raise RuntimeError(f"axon_start_nrt_profile rc={rc}")
        try:
            yield
        finally:
            n = lib.axon_stop_nrt_profile(str(output_dir).encode())
            print(f"ntff profile: {n} file(s) written to {output_dir}", file=sys.stderr)

    state["h"] = _hook


_install_ntff_hook()

import concourse.bass as bass
import concourse.bacc as bacc
import concourse.mybir as mybir
from concourse.tile import TileContext
from concourse.bass_utils import run_bass_kernel_spmd

BF16 = ml_dtypes.bfloat16
FP8 = ml_dtypes.float8_e4m3
F32 = mybir.dt.float32
BF = mybir.dt.bfloat16
F8 = mybir.dt.float8e4

N_CORES = 8
B_FULL = 4096
ROWS = B_FULL // N_CORES  # 512 batch rows per core
S = 4                     # LSTM steps (= image width)
HC = 8                    # hidden chunks of 128 (hidden = 1024)
NG = 4                    # gates (i, f, g, o)
KTB = 4                   # k-tile blocks (of 4 k-tiles each) in the [W;U] stack

LAST_RESULT = None        # stash of BassKernelResults for test harnesses


def build_nc(rows=ROWS, with_bout=True):
    """Build the per-core Bass graph (SPMD: same graph on all 8 cores)."""
    nc = bacc.Bacc()

    # x and the [W;U] stacks are fp8 (e4m3); z matmuls run in DoubleRow mode
    # (2 fp8 weights per PE cell, K=256 per matmul).  wf/wr layout:
    # [pair, k, i, col'] with stack row = pair*256 + i*128 + k and col'
    # j-major-regrouped as before.
    xT_d = nc.declare_dram_parameter("xT", [S, 128, HC, rows], F8, isOutput=False)
    wf_d = nc.declare_dram_parameter("wf", [HC, 128, 8, 2, 512], F8, isOutput=False)
    wr_d = nc.declare_dram_parameter("wr", [HC, 128, 8, 2, 512], F8, isOutput=False)
    biasf_d = nc.declare_dram_parameter("biasf", [128, 32], F32, isOutput=False)
    biasr_d = nc.declare_dram_parameter("biasr", [128, 32], F32, isOutput=False)
    # Conv weights in permuted-hidden order: [p, d, kc, oc] — hidden chunk
    # 2*hh+kc of direction d against output channels oc.
    wout_d = nc.declare_dram_parameter("woutp", [128, 2, 2, 256], BF, isOutput=False)
    bout_d = nc.declare_dram_parameter("bout2", [1, 256], BF, isOutput=False)
    ones_d = nc.declare_dram_parameter("ones1", [1, 128], BF, isOutput=False)
    gam_d = nc.declare_dram_parameter("gam", [128, 2], F32, isOutput=False)
    xres_d = nc.declare_dram_parameter("xres", [rows, 4, 4, 256], F32, isOutput=False)
    out_d = nc.declare_dram_parameter("out", [rows, 4, 4, 256], F32, isOutput=True)

    AF = mybir.ActivationFunctionType
    RC = rows // 128  # row chunks for the conv/epilogue

    with TileContext(nc) as tc:
        with (
            tc.tile_pool(name="const", bufs=1) as const,
            tc.tile_pool(name="w", bufs=10) as wpool,
            tc.tile_pool(name="x", bufs=4) as xpool,
            tc.tile_pool(name="cstate", bufs=2) as cpool,
            tc.tile_pool(name="hstate", bufs=2) as hpool,
            tc.tile_pool(name="gtmp", bufs=16) as gpool,
            tc.tile_pool(name="fin", bufs=6) as fpool,
            tc.tile_pool(name="zps", bufs=8, space="PSUM") as zps,
        ):
            dma = nc.sync.dma_start      # weight stream ring
            dma2 = nc.scalar.dma_start   # x/consts/residual/output ring

            biasf = const.tile([128, 32], F32)
            dma2(out=biasf[:], in_=biasf_d[:])
            biasr = const.tile([128, 32], F32)
            dma2(out=biasr[:], in_=biasr_d[:])
            wout = const.tile([128, 2, 2, 256], BF)
            dma2(out=wout[:], in_=wout_d[:])
            bout = const.tile([1, 256], BF)
            dma2(out=bout[:], in_=bout_d[:])
            ones = const.tile([1, 128], BF)
            dma2(out=ones[:], in_=ones_d[:])
            gam = const.tile([128, 2], F32)
            dma2(out=gam[:], in_=gam_d[:])

            c_prev = [None, None]
            h_prev = [None, None]

            x_tiles = []
            for s in range(S):
                x_sb = xpool.tile([128, HC, rows], F8, tag="x", name=f"x{s}")
                dma2(out=x_sb[:], in_=xT_d[s])
                x_tiles.append(x_sb)

            for s in range(S):
                x_sb = x_tiles[s]

                c0 = cpool.tile([128, HC, rows], F8, tag="c0")
                c1 = cpool.tile([128, HC, rows], F8, tag="c1")
                h0 = hpool.tile([128, HC, rows], BF, tag="h0")
                h1 = hpool.tile([128, HC, rows], BF, tag="h1")
                h_cur = [h0, h1]
                c_cur = [c0, c1]
                npair = 4 if s == 0 else 8  # state is zero at step 0
                # Interleave the two directions at chunk granularity: their
                # chunk-chains are independent, so the cell of one direction
                # overlaps the other's matmuls.
                for j in range(HC):
                    for d in range(2):
                        wd = wf_d if d == 0 else wr_d
                        bias = biasf if d == 0 else biasr
                        c_new = c_cur[d]
                        h_new = h_cur[d]
                        # one DMA stages all K-pair tiles for this hidden chunk
                        wt = wpool.tile([128, 8, 2, 512], F8, tag="w")
                        dma(out=wt[:, :npair, :, :], in_=wd[j][:, :npair, :, :])
                        pst = []
                        for g in range(NG):
                            if s == 0 and g == 1:
                                pst.append(None)
                                continue
                            ps = zps.tile([128, rows], F32, tag="ps")
                            for q in range(npair):
                                if q < 4:
                                    rhs = x_sb[:, 2 * q:2 * q + 2, :]
                                else:
                                    rhs = c_prev[d][:, 2 * (q - 4):2 * (q - 4) + 2, :]
                                nc.tensor.matmul(
                                    ps[:],
                                    wt[:, q, :, g * 128:(g + 1) * 128],
                                    rhs,
                                    start=(q == 0),
                                    stop=(q == npair - 1),
                                    perf_mode=mybir.MatmulPerfMode.DoubleRow,
                                )
                            pst.append(ps)

                        # LSTM cell for hidden chunk j (gate order i,f,g,o).
                        # Faithful to the source bug: the matmul state is the
                        # previous c2, the cell-add state the previous h2.
                        # Sigmoids grouped before tanhs to avoid ACT LUT
                        # reloads; gate tiles bf16 for the DVE 2x mode.
                        si = gpool.tile([128, rows], BF, tag="g")
                        nc.scalar.activation(si[:], pst[0][:], AF.Sigmoid,
                                             bias=bias[:, 0 * 8 + j:0 * 8 + j + 1])
                        so = gpool.tile([128, rows], BF, tag="g")
                        nc.scalar.activation(so[:], pst[3][:], AF.Sigmoid,
                                             bias=bias[:, 3 * 8 + j:3 * 8 + j + 1])
                        if s > 0:
                            sf = gpool.tile([128, rows], BF, tag="g")
                            nc.scalar.activation(sf[:], pst[1][:], AF.Sigmoid,
                                                 bias=bias[:, 1 * 8 + j:1 * 8 + j + 1])
                        tg = gpool.tile([128, rows], BF, tag="g")
                        nc.scalar.activation(tg[:], pst[2][:], AF.Tanh,
                                             bias=bias[:, 2 * 8 + j:2 * 8 + j + 1])
                        if s == 0:
                            nc.vector.tensor_mul(c_new[:, j, :], si[:], tg[:])
                        else:
                            t1 = gpool.tile([128, rows], BF, tag="g")
                            nc.vector.tensor_mul(t1[:], si[:], tg[:])
                            t2 = gpool.tile([128, rows], BF, tag="g")
                            nc.vector.tensor_mul(t2[:], sf[:], h_prev[d][:, j, :])
                            nc.vector.tensor_add(c_new[:, j, :], t1[:], t2[:])
                        tc2 = gpool.tile([128, rows], BF, tag="g")
                        nc.scalar.activation(tc2[:], c_new[:, j, :], AF.Tanh)
                        nc.vector.tensor_mul(h_new[:, j, :], so[:], tc2[:])

                        # Interleave the conv + epilogue for output position
                        # hh = j//2 as soon as both of its hidden chunks are
                        # ready on the second direction's pass.
                        if d == 1 and j % 2 == 1:
                            hh = j // 2
                            for rc in range(RC):
                                pc = zps.tile([128, 256], F32, tag="ps", name="pc")
                                i_mm = 0
                                for dd in range(2):
                                    for kc in range(2):
                                        hsrc = h_new if dd == 1 else h_cur[0]
                                        nc.tensor.matmul(
                                            pc[:],
                                            hsrc[:, 2 * hh + kc, rc * 128:(rc + 1) * 128],
                                            wout[:, dd, kc, :],
                                            start=(i_mm == 0),
                                            stop=(not with_bout and i_mm == 3),
                                        )
                                        i_mm += 1
                                if with_bout:
                                    nc.tensor.matmul(pc[:], ones[0:1, :],
                                                     bout[0:1, :],
                                                     start=False, stop=True)

                                # gamma*lrelu(y) + x = (0.3*gamma)*y
                                #   + (0.7*gamma)*relu(y) + x  — Relu shares
                                # the ACT LUT set with sigmoid/tanh, so no
                                # table reloads anywhere in the kernel.
                                xr = fpool.tile([128, 256], F32, tag="xr")
                                dma2(out=xr[:],
                                    in_=xres_d[rc * 128:(rc + 1) * 128, hh, s, :])
                                rr = fpool.tile([128, 256], F32, tag="rr")
                                nc.scalar.activation(rr[:], pc[:], AF.Relu)
                                bb = fpool.tile([128, 256], F32, tag="bb")
                                nc.vector.scalar_tensor_tensor(
                                    bb[:], pc[:], gam[:, 0:1], xr[:],
                                    mybir.AluOpType.mult, mybir.AluOpType.add,
                                )
                                ot = fpool.tile([128, 256], F32, tag="ot")
                                nc.vector.scalar_tensor_tensor(
                                    ot[:], rr[:], gam[:, 1:2], bb[:],
                                    mybir.AluOpType.mult, mybir.AluOpType.add,
                                )
                                dma2(out=out_d[rc * 128:(rc + 1) * 128, hh, s, :],
                                    in_=ot[:])
                c_prev = c_cur
                h_prev = h_cur

    nc.compile()
    return nc


# Hidden indices are stored permuted on-device: p' = hh*256 + cc, where the
# module's native hidden index is j = cc*4 + hh.  This makes each conv output
# position hh contract over contiguous hidden chunks.
_OLD_OF_NEW = (np.arange(1024) % 256) * 4 + np.arange(1024) // 256


def _pack_wstack(w_top, u_bot):
    """Stack [W; U] (each (1024, 4096) f32), permute U's rows (hidden input)
    and all gate-blocked columns (hidden output) into p' order, regroup
    columns j-major, pack K into DoubleRow pairs -> (8, 128, 2, 4096) fp8."""
    u_bot = u_bot[_OLD_OF_NEW, :]
    w = np.concatenate([w_top, u_bot], 0)
    colperm = np.concatenate([g * 1024 + _OLD_OF_NEW for g in range(NG)])
    w = w[:, colperm]
    w = w.reshape(2048, NG, HC, 128).transpose(0, 2, 1, 3).reshape(2048, 4096)
    # -> [j, k, pair, i, col]: stack row = pair*256 + i*128 + k, col' = j*512+col
    w = w.reshape(8, 2, 128, HC, 512).transpose(3, 2, 0, 1, 4)
    return np.ascontiguousarray(w.astype(FP8))


def _pack_bias(b):
    """(4096,) -> (128, 32): [p, g*8+j] = b_permuted[g*1024 + j*128 + p]."""
    b = np.asarray(b, np.float32).reshape(NG, 1024)[:, _OLD_OF_NEW].reshape(-1)
    return np.ascontiguousarray(
        b.reshape(NG, HC, 128).transpose(2, 0, 1).reshape(128, 32).astype(np.float32))


def _prep_inputs(x, Wf, Uf, bf, Wr, Ur, br, Wout, bout, gamma):
    x = np.asarray(x, np.float32)
    Wf = np.asarray(Wf, np.float32)
    Uf = np.asarray(Uf, np.float32)
    Wr = np.asarray(Wr, np.float32)
    Ur = np.asarray(Ur, np.float32)
    Wout = np.asarray(Wout, np.float32)

    # Reverse direction: rev_in[..., h*256+c] = x[..., h, 255-c]; absorb the
    # flip into Wr's rows so both directions share the same transposed input.
    Wr_perm = Wr.reshape(4, 256, 4096)[:, ::-1].reshape(1024, 4096)

    shared = dict(
        wf=_pack_wstack(Wf, Uf),
        wr=_pack_wstack(Wr_perm, Ur),
        biasf=_pack_bias(bf),
        biasr=_pack_bias(br),
        woutp=np.ascontiguousarray(
            Wout.reshape(2, 2, 128, 256).transpose(2, 0, 1, 3).astype(BF16)),
        bout2=np.ascontiguousarray(np.asarray(bout, np.float32)[None, :].astype(BF16)),
        ones1=np.ones((1, 128), BF16),
        gam=np.ascontiguousarray(np.broadcast_to(
            np.asarray(gamma, np.float32).reshape(-1)[0] * np.array([0.3, 0.7], np.float32),
            (128, 2)).astype(np.float32)),
    )

    xs = x.reshape(N_CORES, ROWS, 4, 4, 256)
    in_maps = []
    for i in range(N_CORES):
        xc = xs[i]
        # fwd_in[s, b, h*256+c] = x[b, h, s, c]  -> transposed, k-tiled bf16.
        xT = xc.transpose(2, 1, 3, 0).reshape(S, 1024, ROWS)
        xTp = np.ascontiguousarray(
            xT.reshape(S, HC, 128, ROWS).transpose(0, 2, 1, 3).astype(FP8))
        m = dict(shared)
        m["xT"] = xTp
        m["xres"] = np.ascontiguousarray(xc)
        in_maps.append(m)
    return in_maps


_NC_CACHE = {}


def kernel(**inputs):
    global LAST_RESULT
    rows = ROWS
    with_bout = bool(np.any(np.asarray(inputs["bout"], np.float32)))
    key = (rows, with_bout)
    if key not in _NC_CACHE:
        _NC_CACHE[key] = build_nc(rows, with_bout)
    nc = _NC_CACHE[key]

    in_maps = _prep_inputs(**inputs)
    res = run_bass_kernel_spmd(
        nc, in_maps, core_ids=list(range(N_CORES)),
        trace=bool(int(os.environ.get("KERNEL_TRACE", "0"))),
    )
    LAST_RESULT = res
    out = np.concatenate([np.asarray(res.results[i]["out"]) for i in range(N_CORES)], 0)
    return out.reshape(B_FULL, 4, 4, 256)


if __name__ == "__main__":
    nc = build_nc()
    print("built ok")


# revision 47
# speedup vs baseline: 1.2045x; 1.2045x over previous
"""Trainium2 Bass kernel for nn_AFRM_1245540516473.

Bidirectional 4-step LSTM (hidden 1024, with the module's state-swap bug)
over (B=4096, 4, 4, 256) NHWC images, followed by a 1x1 conv (512->256),
LeakyReLU(0.3), and residual:  out = gamma * lrelu(conv(cat(hs_f, hs_r))) + x.

Strategy: pure data parallelism over 8 NeuronCores (512 batch rows per
core), weights replicated.  Host pre-packing: [W;U] stacks quantized to
fp8 e4m3 in DoubleRow pair layout with gate columns regrouped, the reverse
direction's channel flip absorbed into Wr's row order, and the hidden
dimension permuted (p' = hh*256+cc) so the 1x1 conv contracts contiguous
partition chunks.  On device, z is computed transposed (gate columns on
partitions, batch rows free) via fp8 DoubleRow matmuls (K=256/matmul)
streaming weights once per step, so the recurrent state leaves the cell in
exactly the layout the next step's matmuls need; the two directions
interleave at chunk granularity, the per-step conv + fused epilogue
(gamma*lrelu(y)+x as (0.3g)y + (0.7g)relu(y) + x — relu shares the ACT LUT
set with sigmoid/tanh, so no table reloads) slot in as soon as their hidden
chunks are ready, and gate/conv PSUM tiles share one 8-bank pool.
"""

import os
import sys

for _p in ("/opt/trn_rl_repo",):
    if _p not in sys.path and os.path.isdir(_p):
        sys.path.insert(0, _p)

import numpy as np
import ml_dtypes


def _install_ntff_hook():
    """Provide antenv.axon_hooks (absent in this image) so
    run_bass_kernel_spmd(trace=True) can capture NTFF profiles through
    libaxon_pjrt.so. No-op when profiling isn't requested or available."""
    import types
    import ctypes
    import contextlib
    try:
        import antenv
    except ImportError:
        return
    if "antenv.axon_hooks" in sys.modules:
        return
    state = {}
    mod = types.ModuleType("antenv.axon_hooks")
    mod.set_axon_ntff_profile_hook = lambda h: state.__setitem__("h", h)
    mod.get_axon_ntff_profile_hook = lambda: state.get("h")
    sys.modules["antenv.axon_hooks"] = mod
    antenv.axon_hooks = mod

    so_path = "/opt/axon/libaxon_pjrt.so"
    if not os.path.exists(so_path):
        return
    lib = ctypes.CDLL(so_path)
    if not hasattr(lib, "axon_start_nrt_profile"):
        return
    lib.axon_start_nrt_profile.argtypes = [ctypes.POINTER(ctypes.c_int64), ctypes.c_size_t]
    lib.axon_start_nrt_profile.restype = ctypes.c_int64
    lib.axon_stop_nrt_profile.argtypes = [ctypes.c_char_p]
    lib.axon_stop_nrt_profile.restype = ctypes.c_int64

    @contextlib.contextmanager
    def _hook(output_dir, device_ids):
        import jax
        jax.devices()
        if device_ids:
            ids = (ctypes.c_int64 * len(device_ids))(*device_ids)
            rc = lib.axon_start_nrt_profile(ids, len(device_ids))
        else:
            rc = lib.axon_start_nrt_profile(None, 0)
        if rc != 0:
            raise RuntimeError(f"axon_start_nrt_profile rc={rc}")
        try:
            yield
        finally:
            n = lib.axon_stop_nrt_profile(str(output_dir).encode())
            print(f"ntff profile: {n} file(s) written to {output_dir}", file=sys.stderr)

    state["h"] = _hook


_install_ntff_hook()

import concourse.bass as bass
import concourse.bacc as bacc
import concourse.mybir as mybir
from concourse.tile import TileContext
from concourse.bass_utils import run_bass_kernel_spmd

BF16 = ml_dtypes.bfloat16
FP8 = ml_dtypes.float8_e4m3
F32 = mybir.dt.float32
BF = mybir.dt.bfloat16
F8 = mybir.dt.float8e4

N_CORES = 8
B_FULL = 4096
ROWS = B_FULL // N_CORES  # 512 batch rows per core
S = 4                     # LSTM steps (= image width)
HC = 8                    # hidden chunks of 128 (hidden = 1024)
NG = 4                    # gates (i, f, g, o)
KTB = 4                   # k-tile blocks (of 4 k-tiles each) in the [W;U] stack

LAST_RESULT = None        # stash of BassKernelResults for test harnesses


def build_nc(rows=ROWS, with_bout=True):
    """Build the per-core Bass graph (SPMD: same graph on all 8 cores)."""
    nc = bacc.Bacc()

    # x and the [W;U] stacks are fp8 (e4m3); z matmuls run in DoubleRow mode
    # (2 fp8 weights per PE cell, K=256 per matmul).  wf/wr layout:
    # [pair, k, i, col'] with stack row = pair*256 + i*128 + k and col'
    # j-major-regrouped as before.
    xT_d = nc.declare_dram_parameter("xT", [S, 128, HC, rows], F8, isOutput=False)
    wf_d = nc.declare_dram_parameter("wf", [HC, 128, 8, 2, 512], F8, isOutput=False)
    wr_d = nc.declare_dram_parameter("wr", [HC, 128, 8, 2, 512], F8, isOutput=False)
    biasf_d = nc.declare_dram_parameter("biasf", [128, 32], F32, isOutput=False)
    biasr_d = nc.declare_dram_parameter("biasr", [128, 32], F32, isOutput=False)
    # Conv weights in permuted-hidden order: [p, d, kc, oc] — hidden chunk
    # 2*hh+kc of direction d against output channels oc.
    wout_d = nc.declare_dram_parameter("woutp", [128, 2, 2, 256], BF, isOutput=False)
    bout_d = nc.declare_dram_parameter("bout2", [1, 256], BF, isOutput=False)
    ones_d = nc.declare_dram_parameter("ones1", [1, 128], BF, isOutput=False)
    gam_d = nc.declare_dram_parameter("gam", [128, 2], F32, isOutput=False)
    xres_d = nc.declare_dram_parameter("xres", [rows, 4, 4, 256], F32, isOutput=False)
    out_d = nc.declare_dram_parameter("out", [rows, 4, 4, 256], F32, isOutput=True)

    AF = mybir.ActivationFunctionType
    RC = rows // 128  # row chunks for the conv/epilogue

    with TileContext(nc) as tc:
        with (
            tc.tile_pool(name="const", bufs=1) as const,
            tc.tile_pool(name="w", bufs=10) as wpool,
            tc.tile_pool(name="x", bufs=4) as xpool,
            tc.tile_pool(name="cstate", bufs=2) as cpool,
            tc.tile_pool(name="hstate", bufs=2) as hpool,
            tc.tile_pool(name="gtmp", bufs=16) as gpool,
            tc.tile_pool(name="fin", bufs=6) as fpool,
            tc.tile_pool(name="zps", bufs=8, space="PSUM") as zps,
        ):
            dma = nc.sync.dma_start      # weight stream ring
            dma2 = nc.scalar.dma_start   # x/consts/residual/output ring

            biasf = const.tile([128, 32], F32)
            dma2(out=biasf[:], in_=biasf_d[:])
            biasr = const.tile([128, 32], F32)
            dma2(out=biasr[:], in_=biasr_d[:])
            wout = const.tile([128, 2, 2, 256], BF)
            dma2(out=wout[:], in_=wout_d[:])
            bout = const.tile([1, 256], BF)
            dma2(out=bout[:], in_=bout_d[:])
            ones = const.tile([1, 128], BF)
            dma2(out=ones[:], in_=ones_d[:])
            gam = const.tile([128, 2], F32)
            dma2(out=gam[:], in_=gam_d[:])

            c_prev = [None, None]
            h_prev = [None, None]

            x_tiles = []
            for s in range(S):
                x_sb = xpool.tile([128, HC, rows], F8, tag="x", name=f"x{s}")
                dma2(out=x_sb[:], in_=xT_d[s])
                x_tiles.append(x_sb)

            for s in range(S):
                x_sb = x_tiles[s]

                c0 = cpool.tile([128, HC, rows], F8, tag="c0")
                c1 = cpool.tile([128, HC, rows], F8, tag="c1")
                h0 = hpool.tile([128, HC, rows], BF, tag="h0")
                h1 = hpool.tile([128, HC, rows], BF, tag="h1")
                h_cur = [h0, h1]
                c_cur = [c0, c1]
                npair = 4 if s == 0 else 8  # state is zero at step 0
                # Interleave the two directions at chunk granularity: their
                # chunk-chains are independent, so the cell of one direction
                # overlaps the other's matmuls.
                for j in range(HC):
                    for d in range(2):
                        wd = wf_d if d == 0 else wr_d
                        bias = biasf if d == 0 else biasr
                        c_new = c_cur[d]
                        h_new = h_cur[d]
                        # one DMA stages all K-pair tiles for this hidden chunk
                        wt = wpool.tile([128, 8, 2, 512], F8, tag="w")
                        dma(out=wt[:, :npair, :, :], in_=wd[j][:, :npair, :, :])
                        pst = []
                        for g in range(NG):
                            if s == 0 and g == 1:
                                pst.append(None)
                                continue
                            ps = zps.tile([128, rows], F32, tag="ps")
                            for q in range(npair):
                                if q < 4:
                                    rhs = x_sb[:, 2 * q:2 * q + 2, :]
                                else:
                                    rhs = c_prev[d][:, 2 * (q - 4):2 * (q - 4) + 2, :]
                                nc.tensor.matmul(
                                    ps[:],
                                    wt[:, q, :, g * 128:(g + 1) * 128],
                                    rhs,
                                    start=(q == 0),
                                    stop=(q == npair - 1),
                                    perf_mode=mybir.MatmulPerfMode.DoubleRow,
                                )
                            pst.append(ps)

                        # LSTM cell for hidden chunk j (gate order i,f,g,o).
                        # Faithful to the source bug: the matmul state is the
                        # previous c2, the cell-add state the previous h2.
                        # Sigmoids grouped before tanhs to avoid ACT LUT
                        # reloads; gate tiles bf16 for the DVE 2x mode.
                        si = gpool.tile([128, rows], BF, tag="g")
                        nc.scalar.activation(si[:], pst[0][:], AF.Sigmoid,
                                             bias=bias[:, 0 * 8 + j:0 * 8 + j + 1])
                        tg = gpool.tile([128, rows], BF, tag="g")
                        nc.scalar.activation(tg[:], pst[2][:], AF.Tanh,
                                             bias=bias[:, 2 * 8 + j:2 * 8 + j + 1])
                        if s > 0:
                            sf = gpool.tile([128, rows], BF, tag="g")
                            nc.scalar.activation(sf[:], pst[1][:], AF.Sigmoid,
                                                 bias=bias[:, 1 * 8 + j:1 * 8 + j + 1])
                        so = gpool.tile([128, rows], BF, tag="g")
                        nc.scalar.activation(so[:], pst[3][:], AF.Sigmoid,
                                             bias=bias[:, 3 * 8 + j:3 * 8 + j + 1])
                        if s == 0:
                            nc.vector.tensor_mul(c_new[:, j, :], si[:], tg[:])
                        else:
                            t1 = gpool.tile([128, rows], BF, tag="g")
                            nc.vector.tensor_mul(t1[:], si[:], tg[:])
                            t2 = gpool.tile([128, rows], BF, tag="g")
                            nc.vector.tensor_mul(t2[:], sf[:], h_prev[d][:, j, :])
                            nc.vector.tensor_add(c_new[:, j, :], t1[:], t2[:])
                        tc2 = gpool.tile([128, rows], BF, tag="g")
                        nc.scalar.activation(tc2[:], c_new[:, j, :], AF.Tanh)
                        nc.vector.tensor_mul(h_new[:, j, :], so[:], tc2[:])

                        # Interleave the conv + epilogue for output position
                        # hh = j//2 as soon as both of its hidden chunks are
                        # ready on the second direction's pass.
                        if d == 1 and j % 2 == 1:
                            hh = j // 2
                            for rc in range(RC):
                                pc = zps.tile([128, 256], F32, tag="ps", name="pc")
                                i_mm = 0
                                for dd in range(2):
                                    for kc in range(2):
                                        hsrc = h_new if dd == 1 else h_cur[0]
                                        nc.tensor.matmul(
                                            pc[:],
                                            hsrc[:, 2 * hh + kc, rc * 128:(rc + 1) * 128],
                                            wout[:, dd, kc, :],
                                            start=(i_mm == 0),
                                            stop=(not with_bout and i_mm == 3),
                                        )
                                        i_mm += 1
                                if with_bout:
                                    nc.tensor.matmul(pc[:], ones[0:1, :],
                                                     bout[0:1, :],
                                                     start=False, stop=True)

                                # gamma*lrelu(y) + x = (0.3*gamma)*y
                                #   + (0.7*gamma)*relu(y) + x  — Relu shares
                                # the ACT LUT set with sigmoid/tanh, so no
                                # table reloads anywhere in the kernel.
                                xr = fpool.tile([128, 256], F32, tag="xr")
                                dma2(out=xr[:],
                                    in_=xres_d[rc * 128:(rc + 1) * 128, hh, s, :])
                                rr = fpool.tile([128, 256], F32, tag="rr")
                                nc.scalar.activation(rr[:], pc[:], AF.Relu)
                                bb = fpool.tile([128, 256], F32, tag="bb")
                                nc.vector.scalar_tensor_tensor(
                                    bb[:], pc[:], gam[:, 0:1], xr[:],
                                    mybir.AluOpType.mult, mybir.AluOpType.add,
                                )
                                ot = fpool.tile([128, 256], F32, tag="ot")
                                nc.vector.scalar_tensor_tensor(
                                    ot[:], rr[:], gam[:, 1:2], bb[:],
                                    mybir.AluOpType.mult, mybir.AluOpType.add,
                                )
                                dma2(out=out_d[rc * 128:(rc + 1) * 128, hh, s, :],
                                    in_=ot[:])
                c_prev = c_cur
                h_prev = h_cur

    nc.compile()
    return nc


# Hidden indices are stored permuted on-device: p' = hh*256 + cc, where the
# module's native hidden index is j = cc*4 + hh.  This makes each conv output
# position hh contract over contiguous hidden chunks.
_OLD_OF_NEW = (np.arange(1024) % 256) * 4 + np.arange(1024) // 256


def _pack_wstack(w_top, u_bot):
    """Stack [W; U] (each (1024, 4096) f32), permute U's rows (hidden input)
    and all gate-blocked columns (hidden output) into p' order, regroup
    columns j-major, pack K into DoubleRow pairs -> (8, 128, 2, 4096) fp8."""
    u_bot = u_bot[_OLD_OF_NEW, :]
    w = np.concatenate([w_top, u_bot], 0)
    colperm = np.concatenate([g * 1024 + _OLD_OF_NEW for g in range(NG)])
    w = w[:, colperm]
    w = w.reshape(2048, NG, HC, 128).transpose(0, 2, 1, 3).reshape(2048, 4096)
    # -> [j, k, pair, i, col]: stack row = pair*256 + i*128 + k, col' = j*512+col
    w = w.reshape(8, 2, 128, HC, 512).transpose(3, 2, 0, 1, 4)
    return np.ascontiguousarray(w.astype(FP8))


def _pack_bias(b):
    """(4096,) -> (128, 32): [p, g*8+j] = b_permuted[g*1024 + j*128 + p]."""
    b = np.asarray(b, np.float32).reshape(NG, 1024)[:, _OLD_OF_NEW].reshape(-1)
    return np.ascontiguousarray(
        b.reshape(NG, HC, 128).transpose(2, 0, 1).reshape(128, 32).astype(np.float32))


def _prep_inputs(x, Wf, Uf, bf, Wr, Ur, br, Wout, bout, gamma):
    x = np.asarray(x, np.float32)
    Wf = np.asarray(Wf, np.float32)
    Uf = np.asarray(Uf, np.float32)
    Wr = np.asarray(Wr, np.float32)
    Ur = np.asarray(Ur, np.float32)
    Wout = np.asarray(Wout, np.float32)

    # Reverse direction: rev_in[..., h*256+c] = x[..., h, 255-c]; absorb the
    # flip into Wr's rows so both directions share the same transposed input.
    Wr_perm = Wr.reshape(4, 256, 4096)[:, ::-1].reshape(1024, 4096)

    shared = dict(
        wf=_pack_wstack(Wf, Uf),
        wr=_pack_wstack(Wr_perm, Ur),
        biasf=_pack_bias(bf),
        biasr=_pack_bias(br),
        woutp=np.ascontiguousarray(
            Wout.reshape(2, 2, 128, 256).transpose(2, 0, 1, 3).astype(BF16)),
        bout2=np.ascontiguousarray(np.asarray(bout, np.float32)[None, :].astype(BF16)),
        ones1=np.ones((1, 128), BF16),
        gam=np.ascontiguousarray(np.broadcast_to(
            np.asarray(gamma, np.float32).reshape(-1)[0] * np.array([0.3, 0.7], np.float32),
            (128, 2)).astype(np.float32)),
    )

    xs = x.reshape(N_CORES, ROWS, 4, 4, 256)
    in_maps = []
    for i in range(N_CORES):
        xc = xs[i]
        # fwd_in[s, b, h*256+c] = x[b, h, s, c]  -> transposed, k-tiled bf16.
        xT = xc.transpose(2, 1, 3, 0).reshape(S, 1024, ROWS)
        xTp = np.ascontiguousarray(
            xT.reshape(S, HC, 128, ROWS).transpose(0, 2, 1, 3).astype(FP8))
        m = dict(shared)
        m["xT"] = xTp
        m["xres"] = np.ascontiguousarray(xc)
        in_maps.append(m)
    return in_maps


_NC_CACHE = {}


def kernel(**inputs):
    global LAST_RESULT
    rows = ROWS
    with_bout = bool(np.any(np.asarray(inputs["bout"], np.float32)))
    key = (rows, with_bout)
    if key not in _NC_CACHE:
        _NC_CACHE[key] = build_nc(rows, with_bout)
    nc = _NC_CACHE[key]

    in_maps = _prep_inputs(**inputs)
    res = run_bass_kernel_spmd(
        nc, in_maps, core_ids=list(range(N_CORES)),
        trace=bool(int(os.environ.get("KERNEL_TRACE", "0"))),
    )
    LAST_RESULT = res
    out = np.concatenate([np.asarray(res.results[i]["out"]) for i in range(N_CORES)], 0)
    return out.reshape(B_FULL, 4, 4, 256)


if __name__ == "__main__":
    nc = build_nc()
    print("built ok")
